# revision 7
# baseline (speedup 1.0000x reference)
"""Trainium2 Bass kernel for sum-of-7-box-blurs (k=3..15, edge padding) * base_map.

Math: out = base_map * sum_k 1/(7 k^2) * V_k(H_k(x)) with V_k/H_k k-wide box
sums (edge padding = clamped indexing, handled by host-side padding).

Horizontal delta decomposition (p = (j-1)/2, d_j = x<<p + x>>p column shifts):
  acc = M_3 x + sum_{j in 3..15 step 2} M_j d_j,   M_j = sum_{k>=j} c_k A_k
where M_j are 15-wide banded vertical matrices folded on the host. Per out
row-tile the vertical mix is a K=128 bf16 matmul accumulating in fp32 PSUM.
d3/d11/d15 are materialized on DVE (even col offsets -> bf16 2x mode), d7/d9
on GPSIMD, and x/d5/d13 are fed as direct column-shifted matmuls (shifts are
free in the rhs access pattern). Matmuls are issued weight-major across each
2048-col window so the PE amortizes weight loads and stays dense (HAM warm).
ScalarE evacuates PSUM; DVE does the base_map multiply in SBUF.

Sharding: rows split across 8 cores; halo rows come from host-side edge
padding so cores are fully independent.
"""

import numpy as np
import ml_dtypes

import concourse.bass as bass
import concourse.mybir as mybir
import concourse.tile as tile
from concourse import bacc, bass_utils

H = W = 4096
NC = 8
RPC = H // NC                 # 512 output rows per core
PAD = 7
PW = W + 2 * PAD              # 4110 padded cols
PR = RPC + 2 * PAD            # 526 padded rows per core
M_TILE = 114                  # valid out rows per PE tile (114 + 14 = 128)
ROW_TILES = [(0, 114), (114, 114), (228, 114), (342, 114), (456, 56)]
CHUNK = 2048                  # column window for arrays + weight-major matmuls
K_SIZES = [3, 5, 7, 9, 11, 13, 15]
BF16 = mybir.dt.bfloat16
F32 = mybir.dt.float32
NP_BF16 = ml_dtypes.bfloat16


def _weights_np() -> np.ndarray:
    """lhsT matrices [7, 128, 128]: lhsT[j][i, m] = w_j[i - m].

    m >= M_TILE columns produce partial sums for out-of-tile rows; they are
    never read. Full 128 weight columns enable fast weight load (FWL)."""
    c = {k: 1.0 / (len(K_SIZES) * k * k) for k in K_SIZES}
    wts = np.zeros((7, 128 + 2 * PAD, 128), dtype=np.float64)
    for ji, j in enumerate(K_SIZES):
        w = np.array(
            [sum(c[k] for k in K_SIZES if k >= j and k >= 2 * abs(d - PAD) + 1)
             for d in range(2 * PAD + 1)])
        for m in range(128):
            wts[ji, m:m + 15, m] = w
    return wts[:, :128, :].astype(NP_BF16)


def _kernel_body(nc, tc, xp_d, bm_d, w_d, out_d):
    add = mybir.AluOpType.add
    mult = mybir.AluOpType.mult

    with (
        tc.tile_pool(name="wpool", bufs=1) as wpool,
        tc.tile_pool(name="xpool", bufs=3) as xpool,
        tc.tile_pool(name="apool", bufs=2) as apool,
        tc.tile_pool(name="bmpool", bufs=2) as bmpool,
        tc.tile_pool(name="ppool", bufs=2) as ppool,
        tc.tile_pool(name="opool", bufs=3) as opool,
        tc.tile_pool(name="psum", bufs=2, space="PSUM") as psum_pool,
    ):
        wsb = wpool.tile([128, 7 * 128], BF16)
        nc.scalar.dma_start(
            out=wsb.rearrange("k (j m) -> k j m", j=7),
            in_=w_d.rearrange("j k m -> k j m"))

        def wt(ji, Krows):
            return wsb[:Krows, ji * 128:(ji + 1) * 128]

        # PE warmup: keep the HAM activity window busy during the initial
        # DMA fill so real matmuls start at full clock.
        warm = psum_pool.tile([128, CHUNK], F32, tag="ps")
        for i in range(48):
            s = i % 4
            nc.tensor.matmul(
                warm[:, s * 512:(s + 1) * 512], wsb[:, 0:128],
                wsb[:, 128:640], start=(i < 4), stop=(i >= 44))

        for rt, Mt in ROW_TILES:
            Krows = min(128, PR - rt)     # 128, last tile 70
            x_sb = xpool.tile([128, PW], BF16, tag="x")
            nc.scalar.dma_start(out=x_sb[:Krows], in_=xp_d[rt:rt + Krows])
            bm_sb = bmpool.tile([128, W], F32, tag="bm")
            nc.scalar.dma_start(out=bm_sb[:Mt], in_=bm_d[rt:rt + Mt])
            X = x_sb[:Krows]

            for co in range(0, W, CHUNK):
                # materialized delta arrays (even col offsets -> DVE 2x mode)
                d3 = apool.tile([128, CHUNK], BF16, tag="d3")
                d11 = apool.tile([128, CHUNK], BF16, tag="d11")
                d15 = apool.tile([128, CHUNK], BF16, tag="d15")
                d7 = apool.tile([128, CHUNK], BF16, tag="d7")
                d9 = apool.tile([128, CHUNK], BF16, tag="d9")
                nc.vector.tensor_tensor(
                    out=d3[:Krows], in0=X[:, co + 6:co + 6 + CHUNK],
                    in1=X[:, co + 8:co + 8 + CHUNK], op=add)
                nc.vector.tensor_tensor(
                    out=d11[:Krows], in0=X[:, co + 2:co + 2 + CHUNK],
                    in1=X[:, co + 12:co + 12 + CHUNK], op=add)
                nc.vector.tensor_tensor(
                    out=d15[:Krows], in0=X[:, co + 0:co + 0 + CHUNK],
                    in1=X[:, co + 14:co + 14 + CHUNK], op=add)
                nc.gpsimd.tensor_tensor(
                    out=d7[:Krows], in0=X[:, co + 4:co + 4 + CHUNK],
                    in1=X[:, co + 10:co + 10 + CHUNK], op=add)
                d9_eng = nc.gpsimd if (co // CHUNK) % 2 == 0 else nc.vector
                d9_eng.tensor_tensor(
                    out=d9[:Krows], in0=X[:, co + 3:co + 3 + CHUNK],
                    in1=X[:, co + 11:co + 11 + CHUNK], op=add)

                ps = psum_pool.tile([128, CHUNK], F32, tag="ps")
                nsl = CHUNK // 512

                def mms(ji, rhs_of, start=False, stop=False):
                    for s in range(nsl):
                        nc.tensor.matmul(
                            ps[:, s * 512:(s + 1) * 512],
                            wt(ji, Krows), rhs_of(s), start=start, stop=stop)

                def xs(s, off):
                    base = co + s * 512 + off
                    return X[:, base:base + 512]

                # weight-major over the window; gpsimd-fed terms last
                mms(0, lambda s: xs(s, 7), start=True)          # x base
                mms(0, lambda s: d3[:Krows, s * 512:s * 512 + 512])
                mms(1, lambda s: xs(s, 5))                      # d5 pair
                mms(1, lambda s: xs(s, 9))
                mms(4, lambda s: d11[:Krows, s * 512:s * 512 + 512])
                mms(5, lambda s: xs(s, 1))                      # d13 pair
                mms(5, lambda s: xs(s, 13))
                mms(6, lambda s: d15[:Krows, s * 512:s * 512 + 512])
                mms(3, lambda s: d9[:Krows, s * 512:s * 512 + 512])
                mms(2, lambda s: d7[:Krows, s * 512:s * 512 + 512],
                    stop=True)

                # evacuate PSUM on ScalarE, multiply by base_map on DVE
                psc = ppool.tile([128, CHUNK], F32, tag="psc")
                nc.scalar.copy(out=psc[:Mt], in_=ps[:Mt])
                osb = opool.tile([128, CHUNK], F32, tag="o")
                nc.vector.tensor_tensor(
                    out=osb[:Mt], in0=psc[:Mt],
                    in1=bm_sb[:Mt, co:co + CHUNK], op=mult)
                nc.sync.dma_start(
                    out=out_d[rt:rt + Mt, co:co + CHUNK], in_=osb[:Mt])


def _build():
    nc = bacc.Bacc("TRN2", target_bir_lowering=False, debug=False)
    xp_d = nc.dram_tensor("xp", [PR, PW], BF16, kind="ExternalInput").ap()
    bm_d = nc.dram_tensor("bm", [RPC, W], F32, kind="ExternalInput").ap()
    w_d = nc.dram_tensor("wts", [7, 128, 128], BF16, kind="ExternalInput").ap()
    out_d = nc.dram_tensor("out", [RPC, W], F32, kind="ExternalOutput").ap()
    with tile.TileContext(nc) as tc:
        _kernel_body(nc, tc, xp_d, bm_d, w_d, out_d)
    nc.compile()
    return nc


_CACHE: dict = {}


def _get_nc():
    if "nc" not in _CACHE:
        _CACHE["nc"] = _build()
    return _CACHE["nc"]


def _in_maps(x: np.ndarray, base_map: np.ndarray) -> list[dict]:
    xp = np.pad(x, PAD, mode="edge").astype(NP_BF16)
    wts = _weights_np()
    maps = []
    for c in range(NC):
        maps.append({
            "xp": np.ascontiguousarray(xp[c * RPC: c * RPC + PR]),
            "bm": np.ascontiguousarray(base_map[c * RPC:(c + 1) * RPC]),
            "wts": wts,
        })
    return maps


def run(x, base_map, **kwargs) -> tuple[np.ndarray, bass_utils.BassKernelResults]:
    x = np.ascontiguousarray(np.asarray(x), dtype=np.float32)
    base_map = np.ascontiguousarray(np.asarray(base_map), dtype=np.float32)
    nc = _get_nc()
    res = bass_utils.run_bass_kernel_spmd(
        nc, _in_maps(x, base_map), core_ids=list(range(NC)), **kwargs)
    out = np.concatenate([r["out"] for r in res.results], axis=0)
    return out[None, None].astype(np.float32, copy=False), res


def kernel(x, base_map) -> np.ndarray:
    return run(x, base_map)[0]


# revision 8
# speedup vs baseline: 1.1767x; 1.1767x over previous
"""Trainium2 Bass kernel for sum-of-7-box-blurs (k=3..15, edge padding) * base_map.

Math: out = base_map * sum_k 1/(7 k^2) * V_k(H_k(x)) with V_k/H_k k-wide box
sums (edge padding = clamped indexing, handled by host-side padding).

Horizontal delta decomposition (p = (j-1)/2, d_j = x<<p + x>>p column shifts):
  acc = M_3 x + sum_{j in 3..15 step 2} M_j d_j,   M_j = sum_{k>=j} c_k A_k
where M_j are 15-wide banded vertical matrices folded on the host. Per out
row-tile the vertical mix is a K=128 bf16 matmul accumulating in fp32 PSUM.
d3/d11/d15 are materialized on DVE (even col offsets -> bf16 2x mode), d7/d9
on GPSIMD, and x/d5/d13 are fed as direct column-shifted matmuls (shifts are
free in the rhs access pattern). Matmuls are issued weight-major across each
2048-col window so the PE amortizes weight loads and stays dense (HAM warm).
ScalarE evacuates PSUM; DVE does the base_map multiply in SBUF.

Sharding: rows split across 8 cores; halo rows come from host-side edge
padding so cores are fully independent.
"""

import numpy as np
import ml_dtypes

import concourse.bass as bass
import concourse.mybir as mybir
import concourse.tile as tile
from concourse import bacc, bass_utils

H = W = 4096
NC = 8
RPC = H // NC                 # 512 output rows per core
PAD = 7
PW = W + 2 * PAD              # 4110 padded cols
PR = RPC + 2 * PAD            # 526 padded rows per core
M_TILE = 114                  # valid out rows per PE tile (114 + 14 = 128)
ROW_TILES = [(0, 114), (114, 114), (228, 114), (342, 114), (456, 56)]
CHUNK = 2048                  # column window for arrays + weight-major matmuls
K_SIZES = [3, 5, 7, 9, 11, 13, 15]
BF16 = mybir.dt.bfloat16
F32 = mybir.dt.float32
NP_BF16 = ml_dtypes.bfloat16


def _weights_np() -> np.ndarray:
    """lhsT matrices [7, 128, 128]: lhsT[j][i, m] = w_j[i - m].

    m >= M_TILE columns produce partial sums for out-of-tile rows; they are
    never read. Full 128 weight columns enable fast weight load (FWL)."""
    c = {k: 1.0 / (len(K_SIZES) * k * k) for k in K_SIZES}
    wts = np.zeros((7, 128 + 2 * PAD, 128), dtype=np.float64)
    for ji, j in enumerate(K_SIZES):
        w = np.array(
            [sum(c[k] for k in K_SIZES if k >= j and k >= 2 * abs(d - PAD) + 1)
             for d in range(2 * PAD + 1)])
        for m in range(128):
            wts[ji, m:m + 15, m] = w
    return wts[:, :128, :].astype(NP_BF16)


def _kernel_body(nc, tc, xp_d, bm_d, w_d, out_d):
    add = mybir.AluOpType.add
    mult = mybir.AluOpType.mult

    with (
        tc.tile_pool(name="wpool", bufs=1) as wpool,
        tc.tile_pool(name="xpool", bufs=3) as xpool,
        tc.tile_pool(name="apool", bufs=2) as apool,
        tc.tile_pool(name="bmpool", bufs=3) as bmpool,
        tc.tile_pool(name="ppool", bufs=2) as ppool,
        tc.tile_pool(name="opool", bufs=3) as opool,
        tc.tile_pool(name="psum", bufs=2, space="PSUM") as psum_pool,
    ):
        wsb = wpool.tile([128, 7 * 128], BF16)
        nc.sync.dma_start(
            out=wsb.rearrange("k (j m) -> k j m", j=7),
            in_=w_d.rearrange("j k m -> k j m"))

        def wt(ji, Krows):
            return wsb[:Krows, ji * 128:(ji + 1) * 128]

        # PE warmup: keep the HAM activity window busy during the initial
        # DMA fill so real matmuls start at full clock.
        warm = psum_pool.tile([128, CHUNK], F32, tag="ps")
        for i in range(48):
            s = i % 4
            nc.tensor.matmul(
                warm[:, s * 512:(s + 1) * 512], wsb[:, 0:128],
                wsb[:, 128:640], start=(i < 4), stop=(i >= 44))

        def load_tile(rt, Mt):
            Krows = min(128, PR - rt)
            x_sb = xpool.tile([128, PW], BF16, tag="x")
            nc.sync.dma_start(out=x_sb[:Krows], in_=xp_d[rt:rt + Krows])
            bm_sb = bmpool.tile([128, W], F32, tag="bm")
            nc.sync.dma_start(out=bm_sb[:Mt], in_=bm_d[rt:rt + Mt])
            return x_sb, bm_sb

        loaded = [load_tile(*ROW_TILES[0]), load_tile(*ROW_TILES[1])]
        for ri, (rt, Mt) in enumerate(ROW_TILES):
            Krows = min(128, PR - rt)     # 128, last tile 70
            x_sb, bm_sb = loaded[ri]
            if ri + 2 < len(ROW_TILES):
                loaded.append(load_tile(*ROW_TILES[ri + 2]))
            X = x_sb[:Krows]

            for co in range(0, W, CHUNK):
                # materialized delta arrays (even col offsets -> DVE 2x mode)
                d3 = apool.tile([128, CHUNK], BF16, tag="d3")
                d11 = apool.tile([128, CHUNK], BF16, tag="d11")
                d15 = apool.tile([128, CHUNK], BF16, tag="d15")
                d7 = apool.tile([128, CHUNK], BF16, tag="d7")
                d9 = apool.tile([128, CHUNK], BF16, tag="d9")
                nc.vector.tensor_tensor(
                    out=d3[:Krows], in0=X[:, co + 6:co + 6 + CHUNK],
                    in1=X[:, co + 8:co + 8 + CHUNK], op=add)
                nc.vector.tensor_tensor(
                    out=d11[:Krows], in0=X[:, co + 2:co + 2 + CHUNK],
                    in1=X[:, co + 12:co + 12 + CHUNK], op=add)
                nc.vector.tensor_tensor(
                    out=d15[:Krows], in0=X[:, co + 0:co + 0 + CHUNK],
                    in1=X[:, co + 14:co + 14 + CHUNK], op=add)
                nc.gpsimd.tensor_tensor(
                    out=d7[:Krows], in0=X[:, co + 4:co + 4 + CHUNK],
                    in1=X[:, co + 10:co + 10 + CHUNK], op=add)
                d9_eng = nc.gpsimd if (co // CHUNK) % 2 == 0 else nc.vector
                d9_eng.tensor_tensor(
                    out=d9[:Krows], in0=X[:, co + 3:co + 3 + CHUNK],
                    in1=X[:, co + 11:co + 11 + CHUNK], op=add)

                ps = psum_pool.tile([128, CHUNK], F32, tag="ps")
                nsl = CHUNK // 512

                def mms(ji, rhs_of, start=False, stop=False):
                    for s in range(nsl):
                        nc.tensor.matmul(
                            ps[:, s * 512:(s + 1) * 512],
                            wt(ji, Krows), rhs_of(s), start=start, stop=stop)

                def xs(s, off):
                    base = co + s * 512 + off
                    return X[:, base:base + 512]

                # weight-major over the window; gpsimd-fed terms last
                mms(0, lambda s: xs(s, 7), start=True)          # x base
                mms(0, lambda s: d3[:Krows, s * 512:s * 512 + 512])
                mms(1, lambda s: xs(s, 5))                      # d5 pair
                mms(1, lambda s: xs(s, 9))
                mms(4, lambda s: d11[:Krows, s * 512:s * 512 + 512])
                mms(5, lambda s: xs(s, 1))                      # d13 pair
                mms(5, lambda s: xs(s, 13))
                mms(6, lambda s: d15[:Krows, s * 512:s * 512 + 512])
                mms(3, lambda s: d9[:Krows, s * 512:s * 512 + 512])
                mms(2, lambda s: d7[:Krows, s * 512:s * 512 + 512],
                    stop=True)

                # evacuate PSUM on ScalarE, multiply by base_map on DVE
                psc = ppool.tile([128, CHUNK], F32, tag="psc")
                nc.scalar.copy(out=psc[:Mt], in_=ps[:Mt])
                osb = opool.tile([128, CHUNK], F32, tag="o")
                nc.vector.tensor_tensor(
                    out=osb[:Mt], in0=psc[:Mt],
                    in1=bm_sb[:Mt, co:co + CHUNK], op=mult)
                nc.sync.dma_start(
                    out=out_d[rt:rt + Mt, co:co + CHUNK], in_=osb[:Mt])


def _build():
    nc = bacc.Bacc("TRN2", target_bir_lowering=False, debug=False)
    xp_d = nc.dram_tensor("xp", [PR, PW], BF16, kind="ExternalInput").ap()
    bm_d = nc.dram_tensor("bm", [RPC, W], F32, kind="ExternalInput").ap()
    w_d = nc.dram_tensor("wts", [7, 128, 128], BF16, kind="ExternalInput").ap()
    out_d = nc.dram_tensor("out", [RPC, W], F32, kind="ExternalOutput").ap()
    with tile.TileContext(nc) as tc:
        _kernel_body(nc, tc, xp_d, bm_d, w_d, out_d)
    nc.compile()
    return nc


_CACHE: dict = {}


def _get_nc():
    if "nc" not in _CACHE:
        _CACHE["nc"] = _build()
    return _CACHE["nc"]


def _in_maps(x: np.ndarray, base_map: np.ndarray) -> list[dict]:
    xp = np.pad(x, PAD, mode="edge").astype(NP_BF16)
    wts = _weights_np()
    maps = []
    for c in range(NC):
        maps.append({
            "xp": np.ascontiguousarray(xp[c * RPC: c * RPC + PR]),
            "bm": np.ascontiguousarray(base_map[c * RPC:(c + 1) * RPC]),
            "wts": wts,
        })
    return maps


def run(x, base_map, **kwargs) -> tuple[np.ndarray, bass_utils.BassKernelResults]:
    x = np.ascontiguousarray(np.asarray(x), dtype=np.float32)
    base_map = np.ascontiguousarray(np.asarray(base_map), dtype=np.float32)
    nc = _get_nc()
    res = bass_utils.run_bass_kernel_spmd(
        nc, _in_maps(x, base_map), core_ids=list(range(NC)), **kwargs)
    out = np.concatenate([r["out"] for r in res.results], axis=0)
    return out[None, None].astype(np.float32, copy=False), res


def kernel(x, base_map) -> np.ndarray:
    return run(x, base_map)[0]


# revision 9
# speedup vs baseline: 1.2321x; 1.0471x over previous
"""Trainium2 Bass kernel for sum-of-7-box-blurs (k=3..15, edge padding) * base_map.

Math: out = base_map * sum_k 1/(7 k^2) * V_k(H_k(x)) with V_k/H_k k-wide box
sums (edge padding = clamped indexing, handled by host-side padding).

Horizontal delta decomposition (p = (j-1)/2, d_j = x<<p + x>>p column shifts):
  acc = M_3 x + sum_{j in 3..15 step 2} M_j d_j,   M_j = sum_{k>=j} c_k A_k
where M_j are 15-wide banded vertical matrices folded on the host. Per out
row-tile the vertical mix is a K=128 bf16 matmul accumulating in fp32 PSUM.
d3/d11/d15 are materialized on DVE (even col offsets -> bf16 2x mode), d7/d9
on GPSIMD, and x/d5/d13 are fed as direct column-shifted matmuls (shifts are
free in the rhs access pattern). Matmuls are issued weight-major across each
2048-col window so the PE amortizes weight loads and stays dense (HAM warm).
ScalarE evacuates PSUM; DVE does the base_map multiply in SBUF.

Sharding: rows split across 8 cores; halo rows come from host-side edge
padding so cores are fully independent.
"""

import numpy as np
import ml_dtypes

import concourse.bass as bass
import concourse.mybir as mybir
import concourse.tile as tile
from concourse import bacc, bass_utils

H = W = 4096
NC = 8
RPC = H // NC                 # 512 output rows per core
PAD = 7
PW = W + 2 * PAD              # 4110 padded cols
PR = RPC + 2 * PAD            # 526 padded rows per core
M_TILE = 114                  # valid out rows per PE tile (114 + 14 = 128)
ROW_TILES = [(0, 114), (114, 114), (228, 114), (342, 114), (456, 56)]
CHUNK = 2048                  # column window for arrays + weight-major matmuls
K_SIZES = [3, 5, 7, 9, 11, 13, 15]
BF16 = mybir.dt.bfloat16
F32 = mybir.dt.float32
NP_BF16 = ml_dtypes.bfloat16


def _weights_np() -> np.ndarray:
    """lhsT matrices [7, 128, 128]: lhsT[j][i, m] = w_j[i - m].

    m >= M_TILE columns produce partial sums for out-of-tile rows; they are
    never read. Full 128 weight columns enable fast weight load (FWL)."""
    c = {k: 1.0 / (len(K_SIZES) * k * k) for k in K_SIZES}
    wts = np.zeros((7, 128 + 2 * PAD, 128), dtype=np.float64)
    for ji, j in enumerate(K_SIZES):
        w = np.array(
            [sum(c[k] for k in K_SIZES if k >= j and k >= 2 * abs(d - PAD) + 1)
             for d in range(2 * PAD + 1)])
        for m in range(128):
            wts[ji, m:m + 15, m] = w
    return wts[:, :128, :].astype(NP_BF16)


def _kernel_body(nc, tc, xp_d, bm_d, w_d, out_d):
    add = mybir.AluOpType.add
    mult = mybir.AluOpType.mult

    with (
        tc.tile_pool(name="wpool", bufs=1) as wpool,
        tc.tile_pool(name="xpool", bufs=3) as xpool,
        tc.tile_pool(name="apool", bufs=2) as apool,
        tc.tile_pool(name="bmpool", bufs=3) as bmpool,
        tc.tile_pool(name="ppool", bufs=2) as ppool,
        tc.tile_pool(name="opool", bufs=3) as opool,
        tc.tile_pool(name="psum", bufs=2, space="PSUM") as psum_pool,
    ):
        wsb = wpool.tile([128, 7 * 128], BF16)
        nc.sync.dma_start(
            out=wsb.rearrange("k (j m) -> k j m", j=7),
            in_=w_d.rearrange("j k m -> k j m"))

        def wt(ji, Krows):
            return wsb[:Krows, ji * 128:(ji + 1) * 128]

        # PE warmup: keep the HAM activity window busy during the initial
        # DMA fill so real matmuls start at full clock.
        warm = psum_pool.tile([128, CHUNK], F32, tag="ps")
        for i in range(48):
            s = i % 4
            nc.tensor.matmul(
                warm[:, s * 512:(s + 1) * 512], wsb[:, 0:128],
                wsb[:, 128:640], start=(i < 4), stop=(i >= 44))

        def load_tile(rt, Mt):
            Krows = min(128, PR - rt)
            x_sb = xpool.tile([128, PW], BF16, tag="x")
            nc.sync.dma_start(out=x_sb[:Krows], in_=xp_d[rt:rt + Krows])
            bm_sb = bmpool.tile([128, W], F32, tag="bm")
            nc.sync.dma_start(out=bm_sb[:Mt], in_=bm_d[rt:rt + Mt])
            return x_sb, bm_sb

        loaded = [load_tile(*ROW_TILES[0]), load_tile(*ROW_TILES[1])]
        for ri, (rt, Mt) in enumerate(ROW_TILES):
            Krows = min(128, PR - rt)     # 128, last tile 70
            x_sb, bm_sb = loaded[ri]
            if ri + 2 < len(ROW_TILES):
                loaded.append(load_tile(*ROW_TILES[ri + 2]))
            X = x_sb[:Krows]

            for co in range(0, W, CHUNK):
                # materialized delta arrays (even col offsets -> DVE 2x mode)
                d3 = apool.tile([128, CHUNK], BF16, tag="d3")
                d11 = apool.tile([128, CHUNK], BF16, tag="d11")
                d15 = apool.tile([128, CHUNK], BF16, tag="d15")
                d7 = apool.tile([128, CHUNK], BF16, tag="d7")
                d9 = apool.tile([128, CHUNK], BF16, tag="d9")
                nc.vector.tensor_tensor(
                    out=d3[:Krows], in0=X[:, co + 6:co + 6 + CHUNK],
                    in1=X[:, co + 8:co + 8 + CHUNK], op=add)
                nc.vector.tensor_tensor(
                    out=d11[:Krows], in0=X[:, co + 2:co + 2 + CHUNK],
                    in1=X[:, co + 12:co + 12 + CHUNK], op=add)
                nc.vector.tensor_tensor(
                    out=d15[:Krows], in0=X[:, co + 0:co + 0 + CHUNK],
                    in1=X[:, co + 14:co + 14 + CHUNK], op=add)
                nc.gpsimd.tensor_tensor(
                    out=d7[:Krows], in0=X[:, co + 4:co + 4 + CHUNK],
                    in1=X[:, co + 10:co + 10 + CHUNK], op=add)
                nc.vector.tensor_tensor(
                    out=d9[:Krows], in0=X[:, co + 3:co + 3 + CHUNK],
                    in1=X[:, co + 11:co + 11 + CHUNK], op=add)

                ps = psum_pool.tile([128, CHUNK], F32, tag="ps")
                nsl = CHUNK // 512

                def mms(ji, rhs_of, start=False, stop=False):
                    for s in range(nsl):
                        nc.tensor.matmul(
                            ps[:, s * 512:(s + 1) * 512],
                            wt(ji, Krows), rhs_of(s), start=start, stop=stop)

                def xs(s, off):
                    base = co + s * 512 + off
                    return X[:, base:base + 512]

                # weight-major over the window; gpsimd-fed terms last
                mms(0, lambda s: xs(s, 7), start=True)          # x base
                mms(0, lambda s: d3[:Krows, s * 512:s * 512 + 512])
                mms(1, lambda s: xs(s, 5))                      # d5 pair
                mms(1, lambda s: xs(s, 9))
                mms(4, lambda s: d11[:Krows, s * 512:s * 512 + 512])
                mms(5, lambda s: xs(s, 1))                      # d13 pair
                mms(5, lambda s: xs(s, 13))
                mms(6, lambda s: d15[:Krows, s * 512:s * 512 + 512])
                mms(3, lambda s: d9[:Krows, s * 512:s * 512 + 512])
                mms(2, lambda s: d7[:Krows, s * 512:s * 512 + 512],
                    stop=True)

                # evacuate PSUM on ScalarE, multiply by base_map on DVE
                psc = ppool.tile([128, CHUNK], F32, tag="psc")
                nc.scalar.copy(out=psc[:Mt], in_=ps[:Mt])
                osb = opool.tile([128, CHUNK], F32, tag="o")
                nc.vector.tensor_tensor(
                    out=osb[:Mt], in0=psc[:Mt],
                    in1=bm_sb[:Mt, co:co + CHUNK], op=mult)
                nc.sync.dma_start(
                    out=out_d[rt:rt + Mt, co:co + CHUNK], in_=osb[:Mt])


def _build():
    nc = bacc.Bacc("TRN2", target_bir_lowering=False, debug=False)
    xp_d = nc.dram_tensor("xp", [PR, PW], BF16, kind="ExternalInput").ap()
    bm_d = nc.dram_tensor("bm", [RPC, W], F32, kind="ExternalInput").ap()
    w_d = nc.dram_tensor("wts", [7, 128, 128], BF16, kind="ExternalInput").ap()
    out_d = nc.dram_tensor("out", [RPC, W], F32, kind="ExternalOutput").ap()
    with tile.TileContext(nc) as tc:
        _kernel_body(nc, tc, xp_d, bm_d, w_d, out_d)
    nc.compile()
    return nc


_CACHE: dict = {}


def _get_nc():
    if "nc" not in _CACHE:
        _CACHE["nc"] = _build()
    return _CACHE["nc"]


def _in_maps(x: np.ndarray, base_map: np.ndarray) -> list[dict]:
    xp = np.pad(x, PAD, mode="edge").astype(NP_BF16)
    wts = _weights_np()
    maps = []
    for c in range(NC):
        maps.append({
            "xp": np.ascontiguousarray(xp[c * RPC: c * RPC + PR]),
            "bm": np.ascontiguousarray(base_map[c * RPC:(c + 1) * RPC]),
            "wts": wts,
        })
    return maps


def run(x, base_map, **kwargs) -> tuple[np.ndarray, bass_utils.BassKernelResults]:
    x = np.ascontiguousarray(np.asarray(x), dtype=np.float32)
    base_map = np.ascontiguousarray(np.asarray(base_map), dtype=np.float32)
    nc = _get_nc()
    res = bass_utils.run_bass_kernel_spmd(
        nc, _in_maps(x, base_map), core_ids=list(range(NC)), **kwargs)
    out = np.concatenate([r["out"] for r in res.results], axis=0)
    return out[None, None].astype(np.float32, copy=False), res


def kernel(x, base_map) -> np.ndarray:
    return run(x, base_map)[0]
